# revision 9
# baseline (speedup 1.0000x reference)
"""Guided channel-wise 3x3 conv (per-pixel weights) on 8 Trainium2 cores.

out[b,c,h,w] = sum_{dh,dw in {-1,0,1}} input[b,c,h+dh,w+dw] * weights[b,c,k(dh,dw),h,w]
with SAME zero padding.  Shapes: input (8,64,128,128) f32,
weights (8,64,9,128,128) f32 -> out (8,64,128,128) f32.

Sharding: pure data parallelism, one batch sample per NeuronCore (B=8 cores).

All device traffic and compute is bf16 (the 2e-2 rel-err budget allows it:
measured end-to-end rel_fro ~ 4.8e-3).  This halves HBM traffic (23.2 MB/core)
and doubles DVE throughput via the 2x_1p perf mode (2-byte dtype, packed
innermost stride).  Host up/down-casts f32<->bf16.

Per-core layout: 128 SBUF partitions = (half, c) with p = half*64 + c; each
partition holds one 64-row half of one channel plane.  The input is pre-padded
on the host into the exact per-partition SBUF layout (66 padded rows x 130
padded cols, zeros on border/halo) and the weights are pre-transposed to
(9, 128, 64*128), so every SBUF tensor is filled by one large contiguous DMA.

Schedule: DVE is the critical path (17 passes x ~4.42us).  The ramp is
DMA-limited, so tap0 is split into quarter-row passes and tap1 into halves,
each gated on the smallest DMA prefix that covers it; the final accumulate is
split into quarters so the output flush overlaps the tail.

Raw bass (no Tile): the walrus build in this container only allows ONE sync
wait per instruction, so all synchronization is explicit standalone wait_ge
instructions + then_inc completions.  SP streams the 9 tap-weight DMAs through
a 4-deep slot ring while DVE runs mult+accumulate per tap.
"""

import numpy as np
import ml_dtypes

from concourse import bass, mybir
from concourse.bass_utils import run_bass_kernel_spmd

B, CI, H, W = 8, 64, 128, 128
K = 9
HH = H // 2  # rows per half-plane (64)
PR = HH + 2  # padded rows per partition (66)
PC = W + 2  # padded cols (130)
NP = 128  # SBUF partitions
FP = HH * W  # free elems per partition of one output half-plane (8192)

BF16 = mybir.dt.bfloat16
NPBF = ml_dtypes.bfloat16

TAPS = [4, 0, 1, 2, 3, 5, 6, 7, 8]  # center tap first: it initializes out
NSLOT = 4  # weight slot ring depth
QR = HH // 4  # 16-row quarter block
QF = FP // 4  # free elems per quarter (2048)
RH = HH // 2
HF = FP // 2


def build_bass():
    nc = bass.Bass()
    inp = nc.declare_dram_parameter("input", [NP, PR * PC], BF16, isOutput=False)
    wts = nc.declare_dram_parameter("weights", [K, NP, FP], BF16, isOutput=False)
    out = nc.declare_dram_parameter("out", [NP, FP], BF16, isOutput=True)

    from contextlib import ExitStack

    with ExitStack() as ctx:
        in_pad = ctx.enter_context(nc.sbuf_tensor("in_pad", [NP, PR * PC], BF16))
        wt_slots = [
            ctx.enter_context(nc.sbuf_tensor(f"wt{i}", [NP, FP], BF16))
            for i in range(NSLOT)
        ]
        tmp = ctx.enter_context(nc.sbuf_tensor("tmp", [NP, FP], BF16))
        out_t = ctx.enter_context(nc.sbuf_tensor("out_t", [NP, FP], BF16))
        block = ctx.enter_context(nc.Block())
        # Exact per-DMA semaphores.  A wait for the full inc-capacity of a
        # semaphore is race-free (cannot be satisfied by shard mixtures of
        # other DMAs); partial-capacity waits are only used where the later
        # DMA on the same semaphore is causally gated on this wait via
        # dve_sem, so it cannot be in flight yet.
        in_a_sem = ctx.enter_context(nc.semaphore("in_a_sem"))
        in_b_sem = ctx.enter_context(nc.semaphore("in_b_sem"))
        w0a_sem = ctx.enter_context(nc.semaphore("w0a_sem"))
        w0b_sem = ctx.enter_context(nc.semaphore("w0b_sem"))
        w1a_sem = ctx.enter_context(nc.semaphore("w1a_sem"))
        w1b_sem = ctx.enter_context(nc.semaphore("w1b_sem"))
        slot_sems = [ctx.enter_context(nc.semaphore(f"slot{i}_sem")) for i in range(NSLOT)]
        flush_sem = ctx.enter_context(nc.semaphore("flush_sem"))
        dve_sem = ctx.enter_context(nc.semaphore("dve_sem"))  # tap consumptions
        out_sem = ctx.enter_context(nc.semaphore("out_sem"))  # final-add quarters

        in3 = in_pad[:].rearrange("p (r w) -> p r w", r=PR)
        out3 = out_t[:].rearrange("p (r w) -> p r w", r=HH)
        tmp3 = tmp[:].rearrange("p (r w) -> p r w", r=HH)

        # DMA order: in_a, wt0_a, in_b, wt0_b, wt1_a, wt1_b, wt2, wt3, then
        # the 4-slot ring for taps 4..8, then the quartered drain.  Taps 2..8
        # stream through slot j%NSLOT; slot_sems[s] is inc'd 16 per DMA into
        # slot s, so the n-th use of a slot gates on 16*n exactly.
        @block.sync
        def _(sync):
            sync.dma_start(out=in_pad[:, 0 : (RH + 2) * PC], in_=inp[:, 0 : (RH + 2) * PC]).then_inc(in_a_sem, 16)
            sync.dma_start(out=wt_slots[0][:, 0:HF], in_=wts[TAPS[0], :, 0:HF]).then_inc(w0a_sem, 16)
            sync.dma_start(out=in_pad[:, (RH + 2) * PC :], in_=inp[:, (RH + 2) * PC :]).then_inc(in_b_sem, 16)
            sync.dma_start(out=wt_slots[0][:, HF:FP], in_=wts[TAPS[0], :, HF:FP]).then_inc(w0b_sem, 16)
            sync.dma_start(out=wt_slots[1][:, 0:HF], in_=wts[TAPS[1], :, 0:HF]).then_inc(w1a_sem, 16)
            sync.dma_start(out=wt_slots[1][:, HF:FP], in_=wts[TAPS[1], :, HF:FP]).then_inc(w1b_sem, 16)
            sync.dma_start(out=wt_slots[2][:], in_=wts[TAPS[2]]).then_inc(slot_sems[2], 16)
            sync.dma_start(out=wt_slots[3][:], in_=wts[TAPS[3]]).then_inc(slot_sems[3], 16)
            for j in range(4, K):
                # slot j%NSLOT was last read by tap j-NSLOT's mult; dve_sem
                # reaches j-NSLOT+1 once that tap's mult is done.
                sync.wait_ge(dve_sem, j - NSLOT + 1)
                sync.dma_start(out=wt_slots[j % NSLOT][:], in_=wts[TAPS[j]]).then_inc(slot_sems[j % NSLOT], 16)
            # Drain: flush output quarters as the final accumulate completes
            # (out_sem is DVE-incremented -> no DMA-ordering assumption).
            for qi in range(4):
                sync.wait_ge(out_sem, qi + 1)
                sync.dma_start(
                    out=out[:, qi * QF : (qi + 1) * QF],
                    in_=out_t[:, qi * QF : (qi + 1) * QF],
                ).then_inc(flush_sem, 16)
            # Retire: full-capacity waits on every semaphore (exact).
            sync.wait_ge(in_a_sem, 16)
            sync.wait_ge(in_b_sem, 16)
            sync.wait_ge(w0a_sem, 16)
            sync.wait_ge(w0b_sem, 16)
            sync.wait_ge(w1a_sem, 16)
            sync.wait_ge(w1b_sem, 16)
            sync.wait_ge(slot_sems[0], 32)  # taps 4, 8
            sync.wait_ge(slot_sems[1], 16)  # tap 5
            sync.wait_ge(slot_sems[2], 32)  # taps 2, 6
            sync.wait_ge(slot_sems[3], 32)  # taps 3, 7
            sync.wait_ge(flush_sem, 64)

        @block.vector
        def _(vector):
            for j, k in enumerate(TAPS):
                dh, dw = k // 3, k % 3
                wt3 = wt_slots[j % NSLOT][:].rearrange("p (r w) -> p r w", r=HH)
                if j == 0:
                    # half-row multiplies gated exactly on (in_a, wt0_a)
                    # and (in_b, wt0_b).
                    vector.wait_ge(in_a_sem, 16)
                    vector.wait_ge(w0a_sem, 16)
                    vector.tensor_tensor(
                        out=out3[:, 0:RH],
                        in0=in3[:, dh : dh + RH, dw : dw + W],
                        in1=wt3[:, 0:RH],
                        op=mybir.AluOpType.mult,
                    )
                    vector.wait_ge(in_b_sem, 16)
                    vector.wait_ge(w0b_sem, 16)
                    vector.tensor_tensor(
                        out=out3[:, RH:HH],
                        in0=in3[:, dh + RH : dh + HH, dw : dw + W],
                        in1=wt3[:, RH:HH],
                        op=mybir.AluOpType.mult,
                    ).then_inc(dve_sem, 1)
                    continue
                if j == 1:
                    # half multiplies gated exactly on the wt1 half DMAs
                    vector.wait_ge(w1a_sem, 16)
                    vector.tensor_tensor(
                        out=tmp3[:, 0:RH],
                        in0=in3[:, dh : dh + RH, dw : dw + W],
                        in1=wt3[:, 0:RH],
                        op=mybir.AluOpType.mult,
                    )
                    vector.wait_ge(w1b_sem, 16)
                    vector.tensor_tensor(
                        out=tmp3[:, RH:HH],
                        in0=in3[:, dh + RH : dh + HH, dw : dw + W],
                        in1=wt3[:, RH:HH],
                        op=mybir.AluOpType.mult,
                    ).then_inc(dve_sem, 1)
                    vector.tensor_tensor(
                        out=out3, in0=out3, in1=tmp3, op=mybir.AluOpType.add
                    )
                    continue
                # taps 2..8: full mult gated exactly on that tap's weight DMA
                # (n-th DMA into this slot -> slot_sems[slot] >= 16*n; later
                # DMAs into the same slot are causally gated on this tap's
                # mult via dve_sem, so a partial-capacity wait is exact).
                vector.wait_ge(slot_sems[j % NSLOT], 16 * (1 if j <= 5 else 2))
                iv = in3[:, dh : dh + HH, dw : dw + W]
                vector.tensor_tensor(
                    out=tmp3, in0=iv, in1=wt3, op=mybir.AluOpType.mult
                ).then_inc(dve_sem, 1)
                if j == len(TAPS) - 1:
                    # quartered final accumulate so the flush overlaps
                    for qi in range(4):
                        vector.tensor_tensor(
                            out=out3[:, qi * QR : (qi + 1) * QR],
                            in0=out3[:, qi * QR : (qi + 1) * QR],
                            in1=tmp3[:, qi * QR : (qi + 1) * QR],
                            op=mybir.AluOpType.add,
                        ).then_inc(out_sem, 1)
                else:
                    vector.tensor_tensor(
                        out=out3, in0=out3, in1=tmp3, op=mybir.AluOpType.add
                    )

    return nc


def _prep_input(x):
    """(64,128,128) f32 -> (128, 66*130) bf16 per-partition padded layout."""
    pad = np.zeros((CI, H + 2, W + 2), dtype=NPBF)
    pad[:, 1 : H + 1, 1 : W + 1] = x.astype(NPBF)
    win = np.stack([pad[:, 0:PR, :], pad[:, HH : HH + PR, :]], axis=0)  # (2,64,66,130)
    return np.ascontiguousarray(win.reshape(NP, PR * PC))


def _prep_weights(w):
    """(64,9,128,128) f32 -> (9, 128, 64*128) bf16 with partition p = half*64 + c."""
    wr = w.reshape(CI, K, 2, HH, W).transpose(1, 2, 0, 3, 4)  # (9,2,64,64,128)
    return np.ascontiguousarray(wr.reshape(K, NP, FP).astype(NPBF))


def _unprep_out(o):
    """(128, 64*128) bf16 -> (64,128,128) f32."""
    return np.ascontiguousarray(
        o.reshape(2, CI, HH, W).transpose(1, 0, 2, 3).reshape(CI, H, W)
    ).astype(np.float32)


_NC = None


def _get_nc():
    global _NC
    if _NC is None:
        _NC = build_bass()
    return _NC


def make_in_maps(input, weights):
    input = np.asarray(input, dtype=np.float32)
    weights = np.asarray(weights, dtype=np.float32)
    return [
        {"input": _prep_input(input[b]), "weights": _prep_weights(weights[b])}
        for b in range(B)
    ]


def kernel(input, weights):
    nc = _get_nc()
    in_maps = make_in_maps(input, weights)
    res = run_bass_kernel_spmd(nc, in_maps, list(range(B)))
    return np.stack([_unprep_out(res.results[b]["out"]) for b in range(B)], axis=0)
